# revision 16
# baseline (speedup 1.0000x reference)
"""Energy-model kernel for Trainium2, data-parallel over 8 NeuronCores.

E[b] = 0.5||x||^2 + 0.5||z||^2 - (phi_vis + phi_enc + phi_bias + phi_pos
       + phi_mem + phi_att)

Per-core layout (128 samples, processed 2 at a time -> 64 blocks):
  - z block (2 samples) lives natively as (128p=(s,p), 256d) fp32; PE
    transposes it to zT (d, (s,p)) which is cast to bf16 for all matmuls.
  - encoder conv (stride == patch size) is an 8x8-patch matmul: phi_enc =
    sum(xp * (z @ Wenc)) with xp the patch-layout view of x.
  - quad/bias terms use 0.5*(v-b)^2 = 0.5 v^2 - v.b + 0.5 b^2; the constant
    0.5 b^2 term is subtracted on the host.
  - memory term: relu(mp) on ACT, then Square-activation with free-axis
    accumulate; attention: exp(gamma*A) accumulated per head, ln, summed.
  - per-block per-partition partials go to accA columns; one mask matmul
    at the end reduces over partitions, separating even/odd samples.
"""
import sys
import types

sys.path.insert(0, "/opt/trn_rl_repo")

import numpy as np
import ml_dtypes

import concourse.bass as bass
import concourse.mybir as mybir
import concourse.tile as tile_mod
import bass_rust
from concourse.tile import TileContext
from concourse.bass_utils import run_bass_kernel_spmd

# ---------------------------------------------------------------- shims
def _split_excess_waits(nc):
    """walrus in this env accepts a single sync wait per instruction, but
    Tile attaches several. Hoist extras onto nop carriers on the same
    engine, placed just before the instruction (engine program order)."""
    cnt = 0
    for f in nc.m.functions:
        for blk in f.blocks:
            il = blk.instructions
            new = []
            for inst in il:
                si = inst.sync_info
                waits = list(si.on_wait or []) if si is not None else []
                if len(waits) > 1:
                    for w in waits[1:]:
                        nop = mybir.InstNoOp(name=f"WSPLIT-{cnt}", ins=[], outs=[])
                        cnt += 1
                        nop.engine = inst.engine
                        nop.sync_info = mybir.SyncInfo(on_wait=[w], on_update=[])
                        new.append(nop)
                    inst.sync_info = mybir.SyncInfo(
                        on_wait=[waits[0]], on_update=list(si.on_update or [])
                    )
                new.append(inst)
            if len(new) != len(il):
                il.clear()
                il.extend(new)
    return cnt


def _install_ntff_hook():
    if "antenv.axon_hooks" in sys.modules:
        return
    mod = types.ModuleType("antenv.axon_hooks")
    state = {"hook": None}
    mod.set_axon_ntff_profile_hook = lambda h: state.__setitem__("hook", h)
    mod.get_axon_ntff_profile_hook = lambda: state["hook"]
    sys.modules["antenv.axon_hooks"] = mod
    try:
        import antenv

        antenv.axon_hooks = mod
        from trn_agent_boot.trn_boot import _ntff_profile_via_ctypes

        mod.set_axon_ntff_profile_hook(
            _ntff_profile_via_ctypes("/opt/axon/libaxon_pjrt.so")
        )
    except Exception:
        pass


_install_ntff_hook()

# ---------------------------------------------------------------- consts
N_CORES = 8
B, C, H = 1024, 3, 64
D, NP, M, NH, R, P = 256, 64, 1024, 8, 32, 8
GAMMA = 0.25
BC = B // N_CORES          # samples per core
NB = BC // 2               # blocks of 2 samples
KCPP = C * P * P           # 192 patch elements
NT = 6                     # partial columns per block
F32 = mybir.dt.float32
BF16 = mybir.dt.bfloat16


def _build_nc(trace_scope=False, nb=NB):
    nc = bass.Bass()
    x_d = nc.dram_tensor("x", [BC, NP, KCPP], F32, kind="ExternalInput")
    z_d = nc.dram_tensor("z", [BC, NP, D], F32, kind="ExternalInput")
    mw_d = nc.dram_tensor("mw", [D, M], BF16, kind="ExternalInput")
    wqk_d = nc.dram_tensor("wqk", [D, 2 * NH * R], BF16, kind="ExternalInput")
    wenc_d = nc.dram_tensor("wenc", [D, KCPP], BF16, kind="ExternalInput")
    vbp_d = nc.dram_tensor("vbp", [128, KCPP], F32, kind="ExternalInput")
    zb_d = nc.dram_tensor("zb", [128, D], F32, kind="ExternalInput")
    ident_d = nc.dram_tensor("ident", [128, 128], F32, kind="ExternalInput")
    mask_d = nc.dram_tensor("mask", [128, 2], F32, kind="ExternalInput")
    out_d = nc.dram_tensor("out", [2, nb], F32, kind="ExternalOutput")

    with TileContext(nc) as tc:
        import contextlib

        with contextlib.ExitStack() as ctx:
            singles = ctx.enter_context(tc.tile_pool(name="singles", bufs=1))
            zpool = ctx.enter_context(tc.tile_pool(name="zpool", bufs=3))
            xpool = ctx.enter_context(tc.tile_pool(name="xpool", bufs=3))
            sbsm = ctx.enter_context(tc.tile_pool(name="sbsm", bufs=3))
            scr = ctx.enter_context(tc.tile_pool(name="scr", bufs=4))
            psZT = ctx.enter_context(tc.tile_pool(name="psZT", bufs=1, space="PSUM"))
            psQK = ctx.enter_context(tc.tile_pool(name="psQK", bufs=2, space="PSUM"))
            psA = ctx.enter_context(tc.tile_pool(name="psA", bufs=2, space="PSUM"))
            psMY = ctx.enter_context(tc.tile_pool(name="psMY", bufs=3, space="PSUM"))

            # constants
            mw_sb = singles.tile([128, 2, M], BF16)
            nc.sync.dma_start(out=mw_sb, in_=mw_d.rearrange("(k p) m -> p k m", p=128))
            wqk_sb = singles.tile([128, 2, 2 * NH * R], BF16)
            nc.sync.dma_start(
                out=wqk_sb, in_=wqk_d.rearrange("(k p) m -> p k m", p=128)
            )
            wenc_sb = singles.tile([128, 2, KCPP], BF16)
            nc.sync.dma_start(
                out=wenc_sb, in_=wenc_d.rearrange("(k p) m -> p k m", p=128)
            )
            vbp_sb = singles.tile([128, KCPP], F32)
            nc.sync.dma_start(out=vbp_sb, in_=vbp_d[:, :])
            zb_sb = singles.tile([128, D], F32)
            nc.sync.dma_start(out=zb_sb, in_=zb_d[:, :])
            ident_sb = singles.tile([128, 128], F32)
            nc.sync.dma_start(out=ident_sb, in_=ident_d[:, :])
            mask_sb = singles.tile([128, 2], F32)
            nc.sync.dma_start(out=mask_sb, in_=mask_d[:, :])

            accA = singles.tile([128, nb, NT], F32)

            # Block-diagonal K holders: bk[buf][g][p=(h',r), s, h', n] is zero
            # except rows 32h'..32h'+31 of plane (s, h'). Zeroed once; gpsimd
            # refreshes only the diagonal blocks each iteration, so A can be
            # computed with k=128 matmuls at base partition 0 (the runtime
            # rejects concurrent row-tiled k=32 matmuls).
            bk = [[None, None], [None, None]]
            for b in range(2):
                for g in range(2):
                    bk_tile = singles.tile(
                        [128, 2, 4, 64], BF16, tag=f"bk{b}{g}", name=f"bk{b}{g}"
                    )
                    bk[b][g] = bk_tile
            for b in range(2):
                for g in range(2):
                    nc.vector.memset(bk[b][g], 0.0)

            for j in range(nb):
                # ---- loads
                z2 = zpool.tile([128, D], F32, tag="z2")
                nc.sync.dma_start(
                    out=z2, in_=z_d[2 * j : 2 * j + 2].rearrange("b p d -> (b p) d")
                )
                xp2 = xpool.tile([128, KCPP], F32, tag="xp")
                nc.sync.dma_start(
                    out=xp2,
                    in_=x_d[2 * j : 2 * j + 2].rearrange("b q k -> (b q) k"),
                )

                # ---- transpose z (PE) then cast to bf16
                zT_ps = psZT.tile([128, 2, 128], F32, tag="zt")
                for kc in range(2):
                    nc.tensor.transpose(
                        zT_ps[:, kc, :], z2[:, 128 * kc : 128 * (kc + 1)], ident_sb
                    )
                zT = sbsm.tile([128, 2, 128], BF16, tag="zt_bf")
                nc.vector.tensor_copy(zT, zT_ps)

                # ---- z quadratic/bias partial: 0.5*sum((z-zb)^2)  [DVE+ACT]
                zdiff = scr.tile([128, D], F32, tag="zdiff")
                nc.vector.tensor_sub(zdiff, z2, zb_sb)
                zsq = scr.tile([128, D], F32, tag="zsq")
                nc.vector.tensor_mul(zsq, zdiff, zdiff)
                nc.vector.tensor_reduce(
                    out=accA[:, j, 0:1], in_=zsq,
                    axis=mybir.AxisListType.X, op=mybir.AluOpType.add,
                )

                # ---- x quadratic/vis partial: 0.5*sum((x-vb)^2)
                xdiff = scr.tile([128, KCPP], F32, tag="xdiff")
                nc.vector.tensor_sub(xdiff, xp2, vbp_sb)
                nc.scalar.activation(
                    out=xdiff, in_=xdiff,
                    func=mybir.ActivationFunctionType.Square,
                    accum_out=accA[:, j, 1:2],
                )

                # ---- y = z @ Wenc ; phi_enc partial = sum(y * xp)
                y_ps = psMY.tile([128, KCPP], F32, tag="my")
                for kc in range(2):
                    nc.tensor.matmul(
                        y_ps, zT[:, kc, :], wenc_sb[:, kc, :],
                        start=(kc == 0), stop=(kc == 1),
                    )
                ymul = scr.tile([128, KCPP], F32, tag="ymul")
                nc.vector.tensor_mul(ymul, y_ps, xp2)
                nc.vector.tensor_reduce(
                    out=accA[:, j, 2:3], in_=ymul,
                    axis=mybir.AxisListType.X, op=mybir.AluOpType.add,
                )

                # ---- memory term: mp = z @ mw, phi_mem = sum(relu(mp)^2)
                for mc in range(2):
                    mp_ps = psMY.tile([128, M // 2], F32, tag="my")
                    for kc in range(2):
                        nc.tensor.matmul(
                            mp_ps, zT[:, kc, :],
                            mw_sb[:, kc, 512 * mc : 512 * (mc + 1)],
                            start=(kc == 0), stop=(kc == 1),
                        )
                    r0 = sbsm.tile([128, M // 2], BF16, tag="r0")
                    nc.scalar.activation(
                        out=r0, in_=mp_ps, func=mybir.ActivationFunctionType.Relu
                    )
                    r0sq = sbsm.tile([128, M // 2], BF16, tag="r0sq")
                    nc.scalar.activation(
                        out=r0sq, in_=r0,
                        func=mybir.ActivationFunctionType.Square,
                        accum_out=accA[:, j, 3 + mc : 4 + mc],
                    )

                # ---- Q,K: out (hr-group, (s,p));  4 groups = Qlo Qhi Klo Khi
                qk_ps = psQK.tile([128, 4, 128], F32, tag="qk")
                for g in range(4):
                    for kc in range(2):
                        nc.tensor.matmul(
                            qk_ps[:, g, :],
                            wqk_sb[:, kc, 128 * g : 128 * (g + 1)],
                            zT[:, kc, :],
                            start=(kc == 0), stop=(kc == 1),
                        )
                qk = sbsm.tile([128, 4, 128], BF16, tag="qk_bf")
                nc.vector.tensor_copy(qk, qk_ps)

                # ---- refresh block-diagonal K, then A = Q^T K (k=128)
                sel = j % 2
                for g in range(2):
                    for s in range(2):
                        for hh in range(4):
                            nc.gpsimd.tensor_copy(
                                out=bk[sel][g][32 * hh : 32 * hh + 32, s, hh, :],
                                in_=qk[32 * hh : 32 * hh + 32, 2 + g,
                                       64 * s : 64 * (s + 1)],
                            )
                a_ps = psA.tile([128, NH, 64], F32, tag="a")
                for s in range(2):
                    for g in range(2):
                        nc.tensor.matmul(
                            a_ps[64 * s : 64 * (s + 1), 4 * g : 4 * g + 4, :],
                            qk[:, g, 64 * s : 64 * (s + 1)],
                            bk[sel][g][:, s, :, :].rearrange("p h n -> p (h n)"),
                            start=True, stop=True,
                        )

                # ---- logsumexp (no max-sub; |gamma*A| < ~6)
                sume = scr.tile([128, NH], F32, tag="sume")
                esc = sbsm.tile([128, 64], BF16, tag="esc")
                for h in range(NH):
                    nc.scalar.activation(
                        out=esc, in_=a_ps[:, h, :],
                        func=mybir.ActivationFunctionType.Exp,
                        scale=GAMMA,
                        accum_out=sume[:, h : h + 1],
                    )
                lns = scr.tile([128, NH], F32, tag="lns")
                nc.scalar.activation(
                    out=lns, in_=sume, func=mybir.ActivationFunctionType.Ln
                )
                nc.vector.tensor_reduce(
                    out=accA[:, j, 5:6], in_=lns,
                    axis=mybir.AxisListType.X, op=mybir.AluOpType.add,
                )

            # ---- final cross-partition reduction:
            # out_ps[g, j, t] = sum_p mask[p, g] * accA[p, j, t]
            fin_ps = psA.tile([2, nb * NT], F32, tag="a")
            nc.tensor.matmul(
                fin_ps, mask_sb, accA.rearrange("p j t -> p (j t)"),
                start=True, stop=True,
            )
            fin_sb = scr.tile([2, nb * NT], F32, tag="fin")
            nc.vector.tensor_copy(fin_sb, fin_ps)
            fin = fin_sb.rearrange("g (j t) -> g j t", t=NT)
            # E_dev = 0.5*(c0+c1) - c2 - (c3+c4) - 4*c5
            t01 = scr.tile([2, nb], F32, tag="f1")
            nc.vector.tensor_add(t01, fin[:, :, 0], fin[:, :, 1])
            t34 = scr.tile([2, nb], F32, tag="f2")
            nc.vector.tensor_add(t34, fin[:, :, 3], fin[:, :, 4])
            t234 = scr.tile([2, nb], F32, tag="f3")
            nc.vector.tensor_add(t234, t34, fin[:, :, 2])
            # r = 0.5*t01 - t234 - 4*c5
            half = scr.tile([2, nb], F32, tag="f4")
            nc.vector.tensor_scalar_mul(half, t01, 0.5)
            att4 = scr.tile([2, nb], F32, tag="f5")
            nc.vector.tensor_scalar_mul(att4, fin[:, :, 5], 4.0)
            s1 = scr.tile([2, nb], F32, tag="f6")
            nc.vector.tensor_add(s1, t234, att4)
            res = scr.tile([2, nb], F32, tag="f7")
            nc.vector.tensor_sub(res, half, s1)
            nc.sync.dma_start(out=out_d[:, :], in_=res)

    _split_excess_waits(nc)
    return nc


_CACHE = {}


def kernel(x, z, encoder_weight, encoder_bias, visible_bias, pos_bias,
           memory_weight, W_Q, W_K):
    x = np.asarray(x, dtype=np.float32)
    z = np.ascontiguousarray(np.asarray(z, dtype=np.float32))
    # im2col staging: (b, c, (i pi), (j pj)) -> (b, (i j), (c pi pj))
    xr = np.ascontiguousarray(
        x.reshape(B, C, 8, P, 8, P).transpose(0, 2, 4, 1, 3, 5).reshape(B, NP, KCPP)
    )
    encoder_weight = np.asarray(encoder_weight, dtype=np.float32)
    encoder_bias = np.asarray(encoder_bias, dtype=np.float32)
    visible_bias = np.asarray(visible_bias, dtype=np.float32)
    pos_bias = np.asarray(pos_bias, dtype=np.float32)
    memory_weight = np.asarray(memory_weight, dtype=np.float32)
    W_Q = np.asarray(W_Q, dtype=np.float32)
    W_K = np.asarray(W_K, dtype=np.float32)

    bf = ml_dtypes.bfloat16
    mw_bf = memory_weight.astype(bf)                                   # (D, M)
    wqk = np.concatenate(
        [
            W_Q.transpose(2, 0, 1).reshape(D, NH * R),
            W_K.transpose(2, 0, 1).reshape(D, NH * R),
        ],
        axis=1,
    ).astype(bf)                                                       # (D, 512)
    wenc = encoder_weight.reshape(D, KCPP).astype(bf)                  # (D, 192)
    # patch-layout visible bias: (c, (i pi), (j pj)) -> ((i j), (c pi pj))
    vbp1 = (
        visible_bias.reshape(C, 8, P, 8, P)
        .transpose(1, 3, 0, 2, 4)
        .reshape(NP, KCPP)
    )
    vbp = np.concatenate([vbp1, vbp1], axis=0).astype(np.float32)      # (128,192)
    zb1 = encoder_bias[None, :] + pos_bias                             # (NP, D)
    zb = np.concatenate([zb1, zb1], axis=0).astype(np.float32)         # (128,256)
    ident = np.eye(128, dtype=np.float32)
    mask = np.zeros((128, 2), dtype=np.float32)
    mask[:64, 0] = 1.0
    mask[64:, 1] = 1.0

    host_corr = 0.5 * float((vbp1.astype(np.float64) ** 2).sum()) + 0.5 * float(
        (zb1.astype(np.float64) ** 2).sum()
    )

    if "nc" not in _CACHE:
        _CACHE["nc"] = _build_nc()
    nc = _CACHE["nc"]

    in_maps = []
    for c in range(N_CORES):
        sl = slice(c * BC, (c + 1) * BC)
        in_maps.append(
            {
                "x": xr[sl],
                "z": z[sl],
                "mw": mw_bf,
                "wqk": wqk,
                "wenc": wenc,
                "vbp": vbp,
                "zb": zb,
                "ident": ident,
                "mask": mask,
            }
        )
    _CACHE["last_in_maps"] = in_maps
    res = run_bass_kernel_spmd(nc, in_maps, list(range(N_CORES)))
    out = np.empty((B,), dtype=np.float32)
    for c in range(N_CORES):
        o = res.results[c]["out"]                 # (2, NB)
        out[c * BC : (c + 1) * BC] = o.T.reshape(BC)
    return (out - np.float32(host_corr)).astype(np.float32)
